# revision 3
# baseline (speedup 1.0000x reference)
"""Trainium2 Bass kernel for 4D convolution (3x3x3x3, pad 1, stride 1).

Problem: x (2, 8, 7, 7, 48, 48) f32, conv (8, 648) f32, bias (8,) f32
         -> out (2, 8, 7, 7, 48, 48) f32.

Sharding: 8 cores = (batch b in {0,1}) x (h-chunk hc in {0..3}, 12 rows).

Per core: two-way row-group concurrent banded matmuls (bf16).

Each core's h-chunk (12 rows) splits into two 6-row sub-chunks:
  LO: outputs t 0..5,  contraction window s 0..7   -> partitions 0..63,  PE rows 0-63
  HI: outputs t 6..11, contraction window s 6..13  -> partitions 64..127, PE rows 64-127
Per (u, shift) the two K=64, M=48 matmuls occupy disjoint row-group pairs of
the PE array and stream concurrently -> ~2x matmul wall-clock vs one
K=112/M=96 matmul. Outputs accumulate in separate PSUM banks (cols 0-47),
drained by two bias-activations per u-row.

Partition layout is s-major (p = s*8 + c); rows s6, s7 are duplicated across
the two halves (128 rows vs 112 unique).
"""

import sys

if "/opt/trn_rl_repo" not in sys.path:
    sys.path.insert(0, "/opt/trn_rl_repo")

import numpy as np
import ml_dtypes

B, C, OC = 2, 8, 8
U, V, H, W = 7, 7, 48, 48
TH = 12
THH = 6             # rows per half
S = TH + 2
SH = THH + 2        # window rows per half
KH = C * SH         # 64  contraction per half
MH = OC * THH       # 48  outputs per half
NCHUNKS = H // TH
NCORES = B * NCHUNKS
NCOL = V * W        # 336
XROW = (V + 2) * (W + 2)  # 450
XFREE = U * XROW

SH_ORDER = [
    (i0, i1, i3) for i0 in (1, 2, 0) for i1 in range(3) for i3 in range(3)
]

N_WARMUP_MM = 4

_built = {}


def _build_nc(reps=None):
    import contextlib

    import concourse.bacc as bacc
    import concourse.mybir as mybir
    from concourse.tile import TileContext

    BF16 = mybir.dt.bfloat16
    F32 = mybir.dt.float32

    nc = bacc.Bacc(
        "TRN2", target_bir_lowering=False, debug=False, num_devices=NCORES
    )
    xw_d = nc.dram_tensor("xw", [128, XFREE], BF16, kind="ExternalInput")
    wt_d = nc.dram_tensor("wt", [128, 27 * MH], BF16, kind="ExternalInput")
    bias_d = nc.dram_tensor("bias", [128, 1], F32, kind="ExternalInput")
    out_d = nc.dram_tensor("out", [2 * MH, U * NCOL], F32, kind="ExternalOutput")

    with TileContext(nc) as tc:
        with (
            tc.tile_pool(name="sbuf", bufs=1) as pool,
            tc.tile_pool(name="psum", bufs=1, space="PSUM") as pp,
        ):
            loop = tc.For_i(0, reps, 1) if reps is not None else contextlib.nullcontext()
            with loop:
                scr = pool.tile([128, 512], BF16, tag="scr")
                nc.gpsimd.memset(scr[:], 0.0)
                ps_w = pp.tile([128, 512], F32, tag="ps_warm")
                for _ in range(N_WARMUP_MM):
                    nc.tensor.matmul(
                        ps_w[:], scr[:, :128], scr[:], start=True, stop=True
                    )

                w_first = pool.tile([128, MH], BF16, tag="wf", name="w_first")
                w_sb = pool.tile([128, 26 * MH], BF16, tag="w", name="w_sb")
                x_sb = pool.tile([128, XFREE], BF16, tag="x", name="x_sb")
                b_sb = pool.tile([128, 1], F32, tag="b")
                nc.scalar.dma_start(out=w_first[:], in_=wt_d[:, 0:MH])
                nc.sync.dma_start(
                    out=x_sb[:, 0 : 3 * XROW], in_=xw_d[:, 0 : 3 * XROW]
                )
                nc.scalar.dma_start(out=w_sb[:], in_=wt_d[:, MH:])
                nc.sync.dma_start(
                    out=x_sb[:, 3 * XROW :], in_=xw_d[:, 3 * XROW :]
                )
                nc.scalar.dma_start(out=b_sb[:], in_=bias_d[:])

                def lhsT_for(pos, half):
                    rows = slice(64 * half, 64 * half + KH)
                    if pos == 0:
                        return w_first[rows, :]
                    return w_sb[rows, (pos - 1) * MH : pos * MH]

                def rhs_for(u, i0, i1, i3, half):
                    return (
                        x_sb[
                            64 * half : 64 * half + KH,
                            (u + i0 - 1) * XROW : (u + i0) * XROW,
                        ]
                        .rearrange("p (v w) -> p v w", v=V + 2)
                        [:, i1 : i1 + V, i3 : i3 + W]
                    )

                ps = [
                    [
                        pp.tile(
                            [MH, NCOL],
                            F32,
                            tag=f"ps{h}",
                            bufs=3,
                            name=f"ps{h}_{u}",
                        )
                        for h in range(2)
                    ]
                    for u in range(U)
                ]
                # halves live at partition bases 0 and 64 (engine operands
                # must sit at 32-aligned bases; 48 is rejected by walrus)
                o_sb = pool.tile([128, U * NCOL], F32, tag="o", name="o_sb")

                for u in range(U):
                    shifts = [
                        (pos, i0, i1, i3)
                        for pos, (i0, i1, i3) in enumerate(SH_ORDER)
                        if 1 <= u + i0 <= 7
                    ]
                    for idx, (pos, i0, i1, i3) in enumerate(shifts):
                        for h in range(2):
                            nc.tensor.matmul(
                                ps[u][h][:],
                                lhsT_for(pos, h),
                                rhs_for(u, i0, i1, i3, h),
                                start=(idx == 0),
                                stop=(idx == len(shifts) - 1),
                            )
                    for h in range(2):
                        nc.scalar.activation(
                            out=o_sb[
                                64 * h : 64 * h + MH,
                                u * NCOL : (u + 1) * NCOL,
                            ],
                            in_=ps[u][h][:],
                            func=mybir.ActivationFunctionType.Identity,
                            bias=b_sb[64 * h : 64 * h + MH, :],
                        )
                    if u == 3:
                        for h in range(2):
                            nc.sync.dma_start(
                                out=out_d[h * MH : (h + 1) * MH, 0 : 4 * NCOL],
                                in_=o_sb[64 * h : 64 * h + MH, 0 : 4 * NCOL],
                            )
                for h in range(2):
                    nc.sync.dma_start(
                        out=out_d[h * MH : (h + 1) * MH, 4 * NCOL :],
                        in_=o_sb[64 * h : 64 * h + MH, 4 * NCOL :],
                    )

    nc.compile()
    return nc


def _get_nc():
    if "nc" not in _built:
        _built["nc"] = _build_nc()
    return _built["nc"]


def _build_weight_inputs(conv, bias):
    Wr = conv.reshape(OC, 3, 3, 3, 3, C).astype(np.float32)
    # wt[p, pos, t_rel*8 + o]; p = 64*half + (s_rel*8 + c); s = s_rel + 6*half
    wt = np.zeros((128, 27, MH), np.float32)
    for half in range(2):
        for t_rel in range(THH):
            for d in range(3):
                s_rel = t_rel + d
                if s_rel >= SH:
                    continue
                for pos, (i0, i1, i3) in enumerate(SH_ORDER):
                    # rows p = 64*half + s_rel*8 + c ; cols t_rel*8 + o
                    p0 = 64 * half + s_rel * 8
                    wt[p0 : p0 + 8, pos, t_rel * 8 : t_rel * 8 + 8] = Wr[
                        :, i0, i1, d, i3, :
                    ].T
    wt = np.ascontiguousarray(
        wt.reshape(128, 27 * MH).astype(ml_dtypes.bfloat16)
    )
    # bias rows: halves at partition bases 0 and 64, (t_rel, o) within
    bias_in = np.zeros((128, 1), np.float32)
    half_bias = np.tile(bias.astype(np.float32), THH).reshape(MH, 1)
    bias_in[0:MH] = half_bias
    bias_in[64 : 64 + MH] = half_bias
    return wt, bias_in


def _build_x_inputs(x):
    xh = np.zeros((B, C, U, V, H + 2, W), np.float32)
    xh[:, :, :, :, 1 : H + 1, :] = x
    xs = []
    for core in range(NCORES):
        b, hc = divmod(core, NCHUNKS)
        slab = xh[b, :, :, :, hc * TH : hc * TH + S, :]  # (C, U, V, S, W)
        xc = np.zeros((C, S, U, V + 2, W + 2), np.float32)
        xc[:, :, :, 1 : V + 1, 1 : W + 1] = slab.transpose(0, 3, 1, 2, 4)
        sm = xc.transpose(1, 0, 2, 3, 4)  # (S, C, U, V+2, W+2)
        x128 = np.empty((128, XFREE), np.float32)
        x128[0:64] = sm[0:SH].reshape(KH, XFREE)
        x128[64:128] = sm[THH : THH + SH].reshape(KH, XFREE)
        xs.append(
            np.ascontiguousarray(x128.astype(ml_dtypes.bfloat16))
        )
    return xs


def kernel(x, conv, bias):
    from concourse.bass_utils import run_bass_kernel_spmd

    nc = _get_nc()
    wt, bias_in = _build_weight_inputs(np.asarray(conv), np.asarray(bias))
    xs = _build_x_inputs(np.asarray(x, dtype=np.float32))
    in_maps = [{"xw": xc, "wt": wt, "bias": bias_in} for xc in xs]
    res = run_bass_kernel_spmd(nc, in_maps, core_ids=list(range(NCORES)))

    out = np.empty((B, OC, U, V, H, W), np.float32)
    for core in range(NCORES):
        b, hc = divmod(core, NCHUNKS)
        # out rows: (half, t_rel, o) -> t = half*6 + t_rel
        r = res.results[core]["out"].reshape(TH, OC, U, V, W)
        out[b, :, :, :, hc * TH : (hc + 1) * TH, :] = r.transpose(1, 2, 3, 0, 4)
    return out
